# revision 6
# baseline (speedup 1.0000x reference)
"""Trainium2 Bass kernel for RoPE linear attention (no softmax, strict causal).

Computes: QR = rope(Q); S = tril(QR @ QR^T, -1); out = S @ V
for Q [B=2, H=8, T=2048, N=1024, D=128], K == Q.

Sharding: B*H = 16 pairs -> 2 per core across 8 cores (fully parallel).

Algorithm (per (b,h)): chunked linear attention with a running prefix
matrix M[n,d] = sum_{s<chunk} QR[s] (x) V[s] held in PSUM (fp32, 2 banks):
  per 256-row chunk c (blocks b0,b1 of 128):
    cross:  out[b] += QR[b] @ M_prefix     (8 matmuls/block, lhsT=QRT chunk)
    local:  P[a,b] = QR[a] @ QR[b]^T for a<=b in chunk (strict diag mask),
            out[b] += P[a,b](as lhsT [s,t]) @ V[a]
    M update: M_k += qd[b][:,k]^T @ V[b]; drain M -> SBUF fp16 for next chunk
This does O(T*N*D) cross work instead of O(T^2*N/2) full scores -- ~2.2x
fewer PE column-cycles than the dense-scores version.

Layouts: qd = cast fp16 + pair de-interleave ([t, evens|odds]); rope on DVE
in [t,n'] with [t,j] tables; QRT = PE-transposed rope'd qd ([n', t]).
"""

import math
import os
import sys

import numpy as np

for _p in ("/opt/trn_rl_repo",):
    if _p not in sys.path and os.path.isdir(_p):
        sys.path.insert(0, _p)

THETA = 2 ** 16
B, H, T, N, D = 2, 8, 2048, 1024, 128
NB = T // 128          # 16 t-blocks
CH = 256               # chunk rows
NCHUNKS = T // CH      # 8 chunks
NC_COUNT = 8
BH_PER_CORE = (B * H) // NC_COUNT  # 2
NPAIR = N // 2         # 512 rotation pairs
NCHUNK = N // 128      # 8 partition chunks of QRT

_cache = {}


def _make_tables():
    """cos/sin tables [T, 512] fp16 (t-major, pair-collapsed even cols)."""
    import jax
    import jax.numpy as jnp

    with jax.default_device(jax.devices("cpu")[0]):
        pos = jnp.floor(jnp.arange(N, dtype=jnp.float32) / 2.0) * 2.0
        freqs = 1.0 / (THETA ** (pos / N)) / (2.0 * math.pi)
        r_phases = jnp.arange(T, dtype=jnp.float32)[:, None] * freqs[None, :]
        ph = (r_phases % 1.0) * (2.0 * math.pi)
        c = np.asarray(jnp.cos(ph))                                  # (T, N)
        s = np.asarray(jnp.sin(ph))
    ct = np.ascontiguousarray(c[:, 0::2]).astype(np.float16)         # (T, 512)
    st = np.ascontiguousarray(s[:, 0::2]).astype(np.float16)
    return ct, st


def _build_nc(repeat=1):
    import concourse.mybir as mybir
    from concourse import bacc
    from concourse.tile import TileContext

    f32 = mybir.dt.float32
    f16 = mybir.dt.float16

    ct_np, st_np = _make_tables()
    # mask[s, t] = 1 if s < t else 0 (keep strictly-upper); the 384-wide
    # variant masks a whole chunk strip [diag b0 | full b0,b1 | diag b1]
    # in one DVE op
    mask_np = np.triu(np.ones((128, 128), np.float16), 1)
    mask384_np = np.concatenate(
        [mask_np, np.ones((128, 128), np.float16), mask_np], axis=1)
    ident_np = np.eye(128, dtype=np.float16)

    nc = bacc.Bacc("TRN2", target_bir_lowering=False, debug=False,
                   num_devices=NC_COUNT)
    q = nc.dram_tensor("q", [BH_PER_CORE, T, N], f32, kind="ExternalInput")
    v = nc.dram_tensor("v", [BH_PER_CORE, T, D], f32, kind="ExternalInput")
    out = nc.dram_tensor("out", [BH_PER_CORE, T, D], f32, kind="ExternalOutput")
    ct_dram = nc.inline_tensor(ct_np, name="ct_tab")
    st_dram = nc.inline_tensor(st_np, name="st_tab")
    mask384_dram = nc.inline_tensor(mask384_np, name="mask384_tab")
    ident_dram = nc.inline_tensor(ident_np, name="ident_tab")

    with TileContext(nc) as tc:
        with tc.tile_pool(name="const", bufs=1) as cpool, \
             tc.tile_pool(name="work", bufs=1) as pool, \
             tc.tile_pool(name="psT", bufs=2, space="PSUM") as psT, \
             tc.tile_pool(name="psS", bufs=2, space="PSUM") as psS, \
             tc.tile_pool(name="psO", bufs=2, space="PSUM") as psO, \
             tc.tile_pool(name="psM", bufs=1, space="PSUM") as psM:

            # tiles only; the DMAs are emitted after pair 0's q load so the
            # constants don't delay the critical head (ident is first
            # needed by pair 0's transposes, mask384 by chunk 0)
            mask384_sb = cpool.tile([128, 384], f16, name="mask384")
            ident_sb = cpool.tile([128, 128], f16, name="ident")

            def load_consts():
                nc.sync.dma_start(out=ident_sb, in_=ident_dram[:, :])
                nc.sync.dma_start(out=mask384_sb, in_=mask384_dram[:, :])
            # [128, 16*512] table tiles: col block tt = table rows of t-tile tt
            ct_sb = cpool.tile([128, NB * NPAIR], f16, name="ct")
            st_sb = cpool.tile([128, NB * NPAIR], f16, name="st")

            def load_tables(lo, hi):
                sl = slice(lo, hi)
                nc.sync.dma_start(
                    out=ct_sb.rearrange("p (a j) -> p a j", a=NB)[:, sl, :],
                    in_=ct_dram.rearrange("(a p) j -> p a j", p=128)[:, sl, :])
                nc.sync.dma_start(
                    out=st_sb.rearrange("p (a j) -> p a j", a=NB)[:, sl, :],
                    in_=st_dram.rearrange("(a p) j -> p a j", p=128)[:, sl, :])

            copy_alt = [0]  # round-robin ACT/DVE for PSUM drains

            def drain_copy(dst, src):
                if copy_alt[0] % 2 == 0:
                    nc.scalar.copy(dst, src)
                else:
                    nc.vector.tensor_copy(out=dst, in_=src)
                copy_alt[0] += 1

            def load_v(bh):
                vf = pool.tile([128, NB * 128], f16, tag="vf", bufs=2,
                               name=f"vf{bh}")
                nc.gpsimd.dma_start(
                    out=vf.rearrange("p (a d) -> p a d", a=NB),
                    in_=v[bh].rearrange("(a p) d -> p a d", p=128),
                )
                return vf

            def load_pair(bh, pp):
                """DMA q tiles 2pp, 2pp+1 into one staging buf, then one
                cast+de-interleave into a pair tile laid out
                [qe(t0) | qe(t1) | qo(t0) | qo(t1)] (512 cols each)."""
                qf = pool.tile([128, 2 * N], f32, tag="qstage", bufs=2,
                               name=f"qf{bh}_{pp}")
                nc.sync.dma_start(
                    out=qf.rearrange("p (a n) -> p a n", a=2),
                    in_=q[bh].rearrange("(a p) n -> p a n", p=128)
                        [:, 2 * pp:2 * pp + 2, :])
                qp = pool.tile([128, 2 * N], f16, tag="qd", bufs=14,
                               name=f"qp{bh}_{pp}")
                # iteration (i=tile, two=parity, j=pair): in (p, i*1024 +
                # j*2 + two) -> out (p, two*1024 + i*512 + j)
                nc.scalar.copy(
                    qp.rearrange("p (two i j) -> p i two j", two=2, i=2),
                    qf.rearrange("p (i j two) -> p i two j", two=2, i=2))
                return qp

            def qd_chunk(qp, i, k):
                """[t, 128] slice of block i (0/1) chunk k in a pair tile."""
                off = (k // 4) * 1024 + i * 512 + (k % 4) * 128
                return qp[:, off:off + 128]

            def load_pair0_fast(bh, qrt_3d):
                """Head fast path for pair 0: per-tile DMA/cast/rope (all
                DVE, half-width ops) and per-tile transposes, so tile 0's
                transposes start ~7us earlier than the batched path."""
                qp = pool.tile([128, 2 * N], f16, tag="qd", bufs=14,
                               name=f"qp{bh}_0f")
                for i in range(2):
                    qf = pool.tile([128, N], f32, tag="qstage0", bufs=2,
                                   name=f"qf0_{bh}_{i}")
                    nc.sync.dma_start(
                        out=qf,
                        in_=q[bh, i * 128:(i + 1) * 128, :])
                    nc.scalar.copy(
                        qp.rearrange("p (two i j) -> p i two j",
                                     two=2, i=2)[:, i],
                        qf.rearrange("p (j two) -> p two j", two=2))
                    qe = qp[:, i * NPAIR:(i + 1) * NPAIR]
                    qo = qp[:, N + i * NPAIR:N + (i + 1) * NPAIR]
                    c_t = ct_sb[:, i * NPAIR:(i + 1) * NPAIR]
                    s_t = st_sb[:, i * NPAIR:(i + 1) * NPAIR]
                    t1 = pool.tile([128, NPAIR], f16, tag="tmp1f", bufs=2,
                                   name=f"t1f_{bh}_{i}")
                    t2 = pool.tile([128, NPAIR], f16, tag="tmp2f", bufs=2,
                                   name=f"t2f_{bh}_{i}")
                    nc.vector.tensor_mul(out=t1, in0=qe, in1=s_t)
                    nc.vector.tensor_mul(out=t2, in0=qo, in1=s_t)
                    nc.vector.tensor_mul(out=qe, in0=qe, in1=c_t)
                    nc.vector.tensor_sub(out=qe, in0=qe, in1=t2)
                    nc.vector.tensor_mul(out=qo, in0=qo, in1=c_t)
                    nc.vector.tensor_add(out=qo, in0=qo, in1=t1)
                    pt = psT.tile([128, 1024], f16, tag="pt",
                                  name=f"ptf{bh}_{i}")
                    for k in range(NCHUNK):
                        nc.tensor.transpose(
                            pt[:, k * 128:(k + 1) * 128],
                            qd_chunk(qp, i, k),
                            ident_sb)
                    drain_copy(
                        qrt_3d[:, :, i * 128:(i + 1) * 128],
                        pt.rearrange("p (c t) -> p c t", c=NCHUNK))
                return qp

            def rope_pair(bh, pp, qp, gp_ops=1):
                """gp_ops = how many of the 4 multiplies go to GPSIMD
                (~2x slower per element than DVE 2x-mode). bh0's fill is
                cadence-critical -> 1; bh1's rope is elastic (runs under
                bh0's compute) -> 3 to rebalance totals."""
                qe, qo = qp[:, :N], qp[:, N:]
                c_t = ct_sb[:, pp * N:(pp + 1) * N]
                s_t = st_sb[:, pp * N:(pp + 1) * N]
                t1 = pool.tile([128, N], f16, tag="tmp1", bufs=2,
                               name=f"t1_{bh}_{pp}")
                t2 = pool.tile([128, N], f16, tag="tmp2", bufs=2,
                               name=f"t2_{bh}_{pp}")
                e1 = nc.gpsimd if gp_ops >= 1 else nc.vector
                e2 = nc.gpsimd if gp_ops >= 2 else nc.vector
                e3 = nc.gpsimd if gp_ops >= 3 else nc.vector
                e1.tensor_mul(out=t1, in0=qe, in1=s_t)
                e2.tensor_mul(out=t2, in0=qo, in1=s_t)
                e3.tensor_mul(out=qe, in0=qe, in1=c_t)
                nc.vector.tensor_sub(out=qe, in0=qe, in1=t2)
                nc.vector.tensor_mul(out=qo, in0=qo, in1=c_t)
                nc.vector.tensor_add(out=qo, in0=qo, in1=t1)

            def transpose_pair(bh, qrt_3d, pp, qp):
                for i in range(2):
                    tt = 2 * pp + i
                    pt = psT.tile([128, 1024], f16, tag="pt",
                                  name=f"pt{bh}_{tt}")
                    for k in range(NCHUNK):
                        nc.tensor.transpose(
                            pt[:, k * 128:(k + 1) * 128],
                            qd_chunk(qp, i, k),
                            ident_sb)
                    drain_copy(
                        qrt_3d[:, :, tt * 128:(tt + 1) * 128],
                        pt.rearrange("p (c t) -> p c t", c=NCHUNK))

            def alloc_qrt(bh):
                qrt_big = pool.tile([128, NCHUNK * T], f16, tag="qrt", bufs=2,
                                    name=f"qrtbig{bh}")
                qrt = [qrt_big[:, k * T:(k + 1) * T] for k in range(NCHUNK)]
                qrt_3d = qrt_big.rearrange("p (c t) -> p c t", c=NCHUNK)
                return qrt, qrt_3d

            def compute_chunk(bh, c, qrt, qds, vf, mps, msb_prev, msb_cur):
                b0, b1 = 2 * c, 2 * c + 1
                # NOTE on start flags: start=True clears the WHOLE bank's
                # has_written bits, so it must be issued exactly once per
                # bank per accumulation epoch (first writer); later groups
                # into fresh regions of the same bank use start=False (their
                # bits are unset -> overwrite+set).
                po = psO.tile([128, 512], f32, tag="po", name=f"po{bh}_{c}")
                # local scores first: they depend only on qrt, so the PE has
                # work at chunk start even while the M drain / msb of the
                # previous chunk is still in flight on ACT.
                sp = psS.tile([128, 512], f32, tag="sp", name=f"sp{bh}_{c}")
                for k in range(NCHUNK):
                    nc.tensor.matmul(
                        sp[:, 0:256],
                        lhsT=qrt[k][:, b0 * 128:(b0 + 1) * 128],
                        rhs=qrt[k][:, b0 * 128:b0 * 128 + 256],
                        start=(k == 0), stop=(k == NCHUNK - 1))
                for k in range(NCHUNK):
                    nc.tensor.matmul(
                        sp[:, 256:384],
                        lhsT=qrt[k][:, b1 * 128:(b1 + 1) * 128],
                        rhs=qrt[k][:, b1 * 128:(b1 + 1) * 128],
                        start=False, stop=(k == NCHUNK - 1))
                # cross: out[b] += QR[b] @ M_prefix
                if c > 0:
                    for bi, b in enumerate((b0, b1)):
                        for k in range(NCHUNK):
                            nc.tensor.matmul(
                                po[:, bi * 128:(bi + 1) * 128],
                                lhsT=qrt[k][:, b * 128:(b + 1) * 128],
                                rhs=msb_prev[:, k * 128:(k + 1) * 128],
                                start=(k == 0 and bi == 0), stop=False)
                strip = pool.tile([128, 384], f16, tag="strip", bufs=2,
                                  name=f"strip{bh}_{c}")
                nc.vector.tensor_mul(out=strip, in0=sp[:, 0:384],
                                     in1=mask384_sb)
                # local AV (start=True only when this is the bank's first
                # writer this chunk, i.e. c == 0 where there is no cross)
                nc.tensor.matmul(po[:, 0:128], lhsT=strip[:, 0:128],
                                 rhs=vf[:, b0 * 128:(b0 + 1) * 128],
                                 start=(c == 0), stop=True)
                nc.tensor.matmul(po[:, 128:256], lhsT=strip[:, 128:256],
                                 rhs=vf[:, b0 * 128:(b0 + 1) * 128],
                                 start=False, stop=False)
                nc.tensor.matmul(po[:, 128:256], lhsT=strip[:, 256:384],
                                 rhs=vf[:, b1 * 128:(b1 + 1) * 128],
                                 start=False, stop=True)
                ob = pool.tile([128, 256], f32, tag="ostage", bufs=2,
                               name=f"ob{bh}_{c}")
                nc.vector.tensor_copy(out=ob, in_=po[:, 0:256])
                nc.sync.dma_start(
                    out=out[bh, c * CH:(c + 1) * CH, :]
                        .rearrange("(bl p) d -> p bl d", p=128),
                    in_=ob.rearrange("p (bl d) -> p bl d", bl=2))
                # M update: M_k += qd[b][:, k]^T @ V[b]; qds holds pair
                # tiles, chunk (b, k) slices via qd_chunk(pair, b%2, k).
                # The last chunk's update/drain has no consumer (there is
                # no chunk c+1 cross) -- skip it; it would sit on the
                # serial tail.
                if c == NCHUNKS - 1:
                    return
                for bi, b in enumerate((b0, b1)):
                    for k in range(NCHUNK):
                        nc.tensor.matmul(
                            mps[:, k * 128:(k + 1) * 128],
                            lhsT=qd_chunk(qds[b // 2], b % 2, k),
                            rhs=vf[:, b * 128:(b + 1) * 128],
                            start=(c == 0 and bi == 0 and k % 4 == 0),
                            stop=(c == NCHUNKS - 2 and bi == 1
                                  and k % 4 == 3))
                # drain M prefix for the next chunk's cross
                nc.scalar.copy(msb_cur, mps)

            for rep in range(repeat):
                # ---- bh0 fill: q data first (it gates rope/transpose/
                # scores); tables ride behind pair 0; vf later (first needed
                # at bh0 compute) ----
                # ---- bh0 fill ahead (pair 0 via the head fast path);
                # chunks 0-2 are emitted into the fill-phase PE gaps
                # (after pairs 1/3/5), the rest of the fill runs ahead ----
                qds0 = []
                qrt0, qrt0_3d = alloc_qrt(0)
                vf0 = vf1 = None
                mps0 = psM.tile([128, 1024], f32, tag="mps",
                                name=f"mps0_{rep}")
                msb0 = [pool.tile([128, 1024], f16, tag=f"msb{i}", bufs=1,
                                  name=f"msb0_{i}_{rep}") for i in range(2)]

                def chunk0(c):
                    compute_chunk(0, c, qrt0, qds0, vf0, mps0,
                                  msb0[(c + 1) % 2], msb0[c % 2])

                for pp in range(NB // 2):
                    if pp == 0:
                        if rep == 0:
                            load_tables(0, 2)  # just pair 0's table rows:
                        if rep == 0:
                            load_consts()   # ident gates the first
                                            # transposes; must precede them
                        qp = load_pair0_fast(0, qrt0_3d)  # gates the head
                        if rep == 0:
                            load_tables(2, 8)
                        qds0.append(qp)
                        continue
                    qp = load_pair(0, pp)
                    if pp == 1:
                        vf0 = load_v(0)
                    if rep == 0 and pp == 3:
                        load_tables(8, 16)
                    if pp == 5:
                        vf1 = load_v(1)
                    rope_pair(0, pp, qp, gp_ops=2)
                    transpose_pair(0, qrt0_3d, pp, qp)
                    qds0.append(qp)
                # bh1 loads + casts (DMA/ACT run during bh0 compute)
                qds1 = []
                for pp in range(NB // 2):
                    qds1.append(load_pair(1, pp))
                qrt1, qrt1_3d = alloc_qrt(1)

                # ---- bh0 compute, bh1 rope+transpose interleaved ----
                for c in range(NCHUNKS):
                    chunk0(c)
                    rope_pair(1, c, qds1[c], gp_ops=2)
                    transpose_pair(1, qrt1_3d, c, qds1[c])

                # ---- bh1 compute (PE-dense tail) ----
                mps1 = psM.tile([128, 1024], f32, tag="mps",
                                name=f"mps1_{rep}")
                msb1 = [pool.tile([128, 1024], f16, tag=f"msb{i}", bufs=1,
                                  name=f"msb1_{i}_{rep}") for i in range(2)]
                for c in range(NCHUNKS):
                    compute_chunk(1, c, qrt1, qds1, vf1, mps1,
                                  msb1[(c + 1) % 2], msb1[c % 2])

    nc.compile()
    return nc


def _get_nc():
    if "nc" not in _cache:
        _cache["nc"] = _build_nc()
    return _cache["nc"]


def kernel(Q, K, V):
    from concourse import bass_utils

    del K  # K is Q by construction
    Qr = np.ascontiguousarray(Q.reshape(B * H, T, N), dtype=np.float32)
    Vr = np.ascontiguousarray(V.reshape(B * H, T, D), dtype=np.float32)

    nc = _get_nc()
    in_maps = []
    for c in range(NC_COUNT):
        lo = c * BH_PER_CORE
        in_maps.append({
            "q": np.ascontiguousarray(Qr[lo:lo + BH_PER_CORE]),
            "v": np.ascontiguousarray(Vr[lo:lo + BH_PER_CORE]),
        })

    res = bass_utils.run_bass_kernel_spmd(
        nc, in_maps, core_ids=list(range(NC_COUNT)),
    )
    _cache["last_result"] = res
    outs = [res.results[c]["out"].reshape(BH_PER_CORE, T, D)
            for c in range(NC_COUNT)]
    return np.concatenate(outs, axis=0).reshape(B, H, T, D).astype(np.float32)
